# revision 5
# baseline (speedup 1.0000x reference)
"""Trainium2 Bass kernel for nn_BAttnDecoderLSTM (batched attention decoder LSTM step).

Strategy: pure data parallel across 8 NeuronCores.  Each core gets 128 batch
rows which map exactly onto the 128 SBUF partitions, so every per-sample
contraction (the batched matvecs of the attention blocks) becomes a fused DVE
op (tensor_tensor_reduce / scalar_tensor_tensor) over the free dimension, and
every shared-weight Linear runs on the TensorEngine with host-pre-transposed
weights.  All large tensors are cast to fp16 on the host (halves HBM traffic,
doubles DVE throughput); accumulations that matter are fp32.
"""

import math
from contextlib import ExitStack

import numpy as np

import concourse.bass as bass
import concourse.mybir as mybir
import concourse.tile as tile
from concourse import bacc
from concourse.bass_utils import run_bass_kernel_spmd
from concourse.masks import make_identity

F16 = mybir.dt.float16
F32 = mybir.dt.float32
AX = mybir.AxisListType.X
OP = mybir.AluOpType
AF_T = mybir.ActivationFunctionType

# Full-size problem dims (hardcoded; the harness always uses these shapes).
DIMS = dict(
    P=128,      # batch rows per core (= SBUF partitions)
    NV=36,      # panoramic views
    F=2052,     # feature dim
    H=512,      # hidden
    E=64,       # action embed
    L=80,       # ctx len
    C=16,       # candidates
    AF=4,       # angle feat
    KS=3,       # shift kernel size
    CH_S=6,     # feature s-chunk
    CH_L=10,    # ctx l-chunk
    CH_C=4,     # cand c-chunk
)
NCORES = 8

_TRACE = False
_LAST_RESULT = None
_CACHE = {}


def _split(n, m=512):
    return [(i, min(m, n - i)) for i in range(0, n, m)]


def _ceil128(n):
    return ((n + 127) // 128) * 128


def build_graph(tc, d, I, O):
    """Build the per-core graph.  I/O are dicts of DRAM APs."""
    nc = tc.nc
    P, NV, F, H, E, L, C, AF, KS = (
        d["P"], d["NV"], d["F"], d["H"], d["E"], d["L"], d["C"], d["AF"], d["KS"])
    CH_S, CH_L, CH_C = d["CH_S"], d["CH_L"], d["CH_C"]
    G = KS
    W = NV // G
    KX = E + F
    KXP = _ceil128(KX)
    NKX = KXP // 128
    NKH = H // 128
    NKC = (3 * H) // 128  # cat = [wctx(2H), h1(H)]
    assert H % 128 == 0 and (3 * H) % 128 == 0 and NV % CH_S == 0
    assert L % CH_L == 0 and C % CH_C == 0 and NV % G == 0

    def softmax_free(in_ap, n, out_tile, tagp):
        """softmax along free dim; in_ap may be PSUM or SBUF (fp32)."""
        mx = work.tile([P, 1], F32, tag=tagp + "_mx")
        nmx = work.tile([P, 1], F32, tag=tagp + "_nmx")
        ssum = work.tile([P, 1], F32, tag=tagp + "_sum")
        rec = work.tile([P, 1], F32, tag=tagp + "_rec")
        ex = work.tile([P, n], F32, tag=tagp + "_exp")
        nc.vector.reduce_max(mx, in_ap, axis=AX)
        nc.vector.tensor_scalar_mul(nmx, mx, -1.0)
        nc.scalar.activation(ex, in_ap, AF_T.Exp, bias=nmx, scale=1.0,
                             accum_out=ssum)
        nc.vector.reciprocal(rec, ssum)
        nc.vector.tensor_scalar_mul(out_tile, ex, rec)

    with ExitStack() as stk:
        const = stk.enter_context(tc.tile_pool(name="const", bufs=1))
        work = stk.enter_context(tc.tile_pool(name="work", bufs=1))
        psmm = stk.enter_context(tc.tile_pool(name="psmm", bufs=2, space="PSUM"))
        pstr = stk.enter_context(tc.tile_pool(name="pstr", bufs=2, space="PSUM"))

        # ---------------- constants & small state ----------------
        ident = const.tile([128, 128], F16)
        make_identity(nc, ident)
        ones = const.tile([1, 128], F16)
        nc.vector.memset(ones, 1.0)

        act_sb = const.tile([P, AF], F16)
        nc.sync.dma_start(out=act_sb, in_=I["action16"])
        ph1_sb = const.tile([P, H], F16)
        nc.sync.dma_start(out=ph1_sb, in_=I["prevh16"])
        c0_sb = const.tile([P, H], F32)
        nc.sync.dma_start(out=c0_sb, in_=I["c0_32"])
        maskb_sb = const.tile([P, L], F32)
        nc.sync.dma_start(out=maskb_sb, in_=I["maskb32"])
        wemb_sb = const.tile([AF, E], F16)
        nc.sync.dma_start(out=wemb_sb, in_=I["wembT"])
        bemb_sb = const.tile([1, E], F16)
        nc.sync.dma_start(out=bemb_sb, in_=I["bemb"])
        bsh_sb = const.tile([1, KS], F16)
        nc.sync.dma_start(out=bsh_sb, in_=I["bsh"])
        bihhh_sb = const.tile([1, 4 * H], F16)
        nc.sync.dma_start(out=bihhh_sb, in_=I["bihhh"])

        # transpose prev_h1 -> [H, P] chunks (lhsT for all prev_h1 matmuls)
        ph1T = work.tile([128, NKH, 128], F16)
        for k in range(NKH):
            ps = pstr.tile([128, 128], F16, tag="tr")
            nc.tensor.transpose(ps, ph1_sb[:, k * 128:(k + 1) * 128], ident)
            nc.scalar.copy(ph1T[:, k, :], ps)

        # ---------------- phase A: shift-attn linear_in + action embed ----
        stA = ExitStack()
        wfp = stA.enter_context(tc.tile_pool(name="wfp", bufs=1))
        wf_sb = wfp.tile([128, NKH, F], F16)
        nc.sync.dma_start(out=wf_sb, in_=I["wfT"].rearrange("(k p) n -> p k n", p=128))
        wsh_sb = wfp.tile([128, NKH, KS], F16)
        nc.sync.dma_start(out=wsh_sb, in_=I["wshT"].rearrange("(k p) n -> p k n", p=128))

        # tgt = prev_h1 @ Win_f.T   [P, F] fp16
        tgt16 = work.tile([P, F], F16)
        for (ns, nw) in _split(F):
            ps = psmm.tile([128, 512], F32, tag="mm")
            for k in range(NKH):
                nc.tensor.matmul(ps[:, :nw], ph1T[:, k, :], wf_sb[:, k, ns:ns + nw],
                                 start=(k == 0), stop=(k == NKH - 1))
            nc.scalar.copy(tgt16[:, ns:ns + nw], ps[:, :nw])

        # a = tanh(action @ W_emb.T + b_emb) -> x[:, 0:E]
        x = work.tile([P, KXP], F16)
        if KXP > KX:
            nc.vector.memset(x[:, KX:], 0.0)
        psT = pstr.tile([128, 128], F16, tag="tr")
        nc.tensor.transpose(psT[:AF, :], act_sb, ident)
        actT = work.tile([AF, 128], F16)
        nc.scalar.copy(actT, psT[:AF, :])
        psa = psmm.tile([128, 512], F32, tag="mm")
        nc.tensor.matmul(psa[:, :E], actT, wemb_sb, start=True, stop=False)
        nc.tensor.matmul(psa[:, :E], ones, bemb_sb, start=False, stop=True)
        nc.scalar.activation(x[:, 0:E], psa[:, :E], AF_T.Tanh)

        # kern = softmax(prev_h1 @ Wsh.T + bsh)  [P, KS]
        psk = psmm.tile([128, 512], F32, tag="mm")
        for k in range(NKH):
            nc.tensor.matmul(psk[:, :KS], ph1T[:, k, :], wsh_sb[:, k, :],
                             start=(k == 0), stop=False)
        nc.tensor.matmul(psk[:, :KS], ones, bsh_sb, start=False, stop=True)
        kern_t = work.tile([P, KS], F32)
        softmax_free(psk[:, :KS], KS, kern_t, "k")

        # ---------------- phase B: scores over panoramic features --------
        stB = ExitStack()
        featp = stB.enter_context(tc.tile_pool(name="featp", bufs=3))
        scores = work.tile([P, NV], F32)
        scr_f = work.tile([P, F], F16)
        for c in range(NV // CH_S):
            ft = featp.tile([P, CH_S, F], F16, tag="feat")
            nc.sync.dma_start(out=ft, in_=I["feat16"][:, c * CH_S:(c + 1) * CH_S, :])
            for j in range(CH_S):
                s = c * CH_S + j
                nc.vector.scalar_tensor_tensor(
                    out=scr_f, in0=ft[:, j, :], scalar=1.0, in1=tgt16,
                    op0=OP.bypass, op1=OP.mult, accum_out=scores[:, s:s + 1])

        # softmax + per-sample circular shift conv
        attn = work.tile([P, NV], F32)
        softmax_free(scores, NV, attn, "s")
        attn3 = attn.rearrange("p (g w) -> p g w", w=W)
        pad = work.tile([P, G, W + 2], F32)
        nc.vector.tensor_copy(pad[:, :, 1:W + 1], attn3)
        nc.vector.tensor_copy(pad[:, :, 0:1], attn3[:, :, W - 1:W])
        nc.vector.tensor_copy(pad[:, :, W + 1:W + 2], attn3[:, :, 0:1])
        attn_s = work.tile([P, G, W], F32)
        nc.vector.tensor_scalar_mul(attn_s, pad[:, :, 0:W], kern_t[:, 0:1])
        for k in range(1, KS):
            nc.vector.scalar_tensor_tensor(
                out=attn_s, in0=pad[:, :, k:k + W], scalar=kern_t[:, k:k + 1],
                in1=attn_s, op0=OP.mult, op1=OP.add)
        attn_flat = attn_s.rearrange("p g w -> p (g w)")

        # attn_feat = einsum('bs,bsf->bf') -> x[:, E:E+F] (second feature pass)
        for c in range(NV // CH_S):
            ft = featp.tile([P, CH_S, F], F16, tag="feat")
            nc.sync.dma_start(out=ft, in_=I["feat16"][:, c * CH_S:(c + 1) * CH_S, :])
            for j in range(CH_S):
                s = c * CH_S + j
                if s == 0:
                    nc.vector.tensor_scalar_mul(x[:, E:E + F], ft[:, j, :],
                                                attn_flat[:, 0:1])
                else:
                    nc.vector.scalar_tensor_tensor(
                        out=x[:, E:E + F], in0=ft[:, j, :],
                        scalar=attn_flat[:, s:s + 1], in1=x[:, E:E + F],
                        op0=OP.mult, op1=OP.add)
        stB.close()
        stA.close()

        # ---------------- phase C: LSTM cell ------------------------------
        stC = ExitStack()
        lstp = stC.enter_context(tc.tile_pool(name="lstp", bufs=1))
        wih_sb = lstp.tile([128, NKX, 4 * H], F16)
        nc.sync.dma_start(out=wih_sb, in_=I["wihT"].rearrange("(k p) n -> p k n", p=128))
        whh_sb = lstp.tile([128, NKH, 4 * H], F16)
        nc.sync.dma_start(out=whh_sb, in_=I["whhT"].rearrange("(k p) n -> p k n", p=128))

        xT = work.tile([128, NKX, 128], F16)
        for k in range(NKX):
            ps = pstr.tile([128, 128], F16, tag="tr")
            nc.tensor.transpose(ps, x[:, k * 128:(k + 1) * 128], ident)
            nc.scalar.copy(xT[:, k, :], ps)

        # gate order (PyTorch): i, f, g, o ; each n-tile is one gate (H<=512)
        gfuncs = [AF_T.Sigmoid, AF_T.Sigmoid, AF_T.Tanh, AF_T.Sigmoid]
        gsb = [work.tile([P, H], F32, tag=f"g{i}", name=f"g{i}") for i in range(4)]
        for g in range(4):
            ps = psmm.tile([128, 512], F32, tag="mm")
            for k in range(NKX):
                nc.tensor.matmul(ps[:, :H], xT[:, k, :], wih_sb[:, k, g * H:(g + 1) * H],
                                 start=(k == 0), stop=False)
            for k in range(NKH):
                nc.tensor.matmul(ps[:, :H], ph1T[:, k, :], whh_sb[:, k, g * H:(g + 1) * H],
                                 start=False, stop=False)
            nc.tensor.matmul(ps[:, :H], ones, bihhh_sb[:, g * H:(g + 1) * H],
                             start=False, stop=True)
            nc.scalar.activation(gsb[g], ps[:, :H], gfuncs[g])
        stC.close()

        t1 = work.tile([P, H], F32)
        t2 = work.tile([P, H], F32)
        c1 = work.tile([P, H], F32)
        h1 = work.tile([P, H], F32)
        nc.vector.tensor_tensor(t1, gsb[1], c0_sb, OP.mult)
        nc.vector.tensor_tensor(t2, gsb[0], gsb[2], OP.mult)
        nc.vector.tensor_tensor(c1, t1, t2, OP.add)
        tanh_c1 = work.tile([P, H], F32)
        nc.scalar.activation(tanh_c1, c1, AF_T.Tanh)
        nc.vector.tensor_tensor(h1, gsb[3], tanh_c1, OP.mult)
        h1_16 = work.tile([P, H], F16)
        nc.scalar.copy(h1_16, h1)
        nc.sync.dma_start(out=O["h1"], in_=h1)
        nc.sync.dma_start(out=O["c1"], in_=c1)

        # ---------------- phase D: ctx attention --------------------------
        stD = ExitStack()
        ctxw = stD.enter_context(tc.tile_pool(name="ctxw", bufs=1))
        winc_sb = ctxw.tile([128, NKH, 2 * H], F16)
        nc.sync.dma_start(out=winc_sb, in_=I["wincT"].rearrange("(k p) n -> p k n", p=128))
        woutc_sb = ctxw.tile([128, NKC, H], F16)
        nc.sync.dma_start(out=woutc_sb, in_=I["woutcT"].rearrange("(k p) n -> p k n", p=128))
        wink_sb = ctxw.tile([128, NKH, F], F16)
        nc.sync.dma_start(out=wink_sb, in_=I["winkT"].rearrange("(k p) n -> p k n", p=128))

        h1T = work.tile([128, NKH, 128], F16)
        for k in range(NKH):
            ps = pstr.tile([128, 128], F16, tag="tr")
            nc.tensor.transpose(ps, h1_16[:, k * 128:(k + 1) * 128], ident)
            nc.scalar.copy(h1T[:, k, :], ps)

        tgtc16 = work.tile([P, 2 * H], F16)
        for (ns, nw) in _split(2 * H):
            ps = psmm.tile([128, 512], F32, tag="mm")
            for k in range(NKH):
                nc.tensor.matmul(ps[:, :nw], h1T[:, k, :], winc_sb[:, k, ns:ns + nw],
                                 start=(k == 0), stop=(k == NKH - 1))
            nc.scalar.copy(tgtc16[:, ns:ns + nw], ps[:, :nw])

        sc = work.tile([P, L], F32)
        scr_c = work.tile([P, 2 * H], F16)
        stE = ExitStack()
        ctxp = stE.enter_context(tc.tile_pool(name="ctxp", bufs=3))
        for c in range(L // CH_L):
            ct = ctxp.tile([P, CH_L, 2 * H], F16, tag="ctx")
            nc.sync.dma_start(out=ct, in_=I["ctx16"][:, c * CH_L:(c + 1) * CH_L, :])
            for j in range(CH_L):
                ll = c * CH_L + j
                nc.vector.scalar_tensor_tensor(
                    out=scr_c, in0=ct[:, j, :], scalar=1.0, in1=tgtc16,
                    op0=OP.bypass, op1=OP.mult, accum_out=sc[:, ll:ll + 1])
        nc.vector.tensor_tensor(sc, sc, maskb_sb, OP.add)

        alpha = work.tile([P, L], F32)
        softmax_free(sc, L, alpha, "c")

        cat = work.tile([P, 3 * H], F16)
        nc.scalar.copy(cat[:, 2 * H:3 * H], h1)
        for c in range(L // CH_L):
            ct = ctxp.tile([P, CH_L, 2 * H], F16, tag="ctx")
            nc.sync.dma_start(out=ct, in_=I["ctx16"][:, c * CH_L:(c + 1) * CH_L, :])
            for j in range(CH_L):
                ll = c * CH_L + j
                if ll == 0:
                    nc.vector.tensor_scalar_mul(cat[:, 0:2 * H], ct[:, j, :],
                                                alpha[:, 0:1])
                else:
                    nc.vector.scalar_tensor_tensor(
                        out=cat[:, 0:2 * H], in0=ct[:, j, :],
                        scalar=alpha[:, ll:ll + 1], in1=cat[:, 0:2 * H],
                        op0=OP.mult, op1=OP.add)
        stE.close()

        # h_tilde = tanh(cat @ Wout_c.T)
        catT = work.tile([128, NKC, 128], F16)
        for k in range(NKC):
            ps = pstr.tile([128, 128], F16, tag="tr")
            nc.tensor.transpose(ps, cat[:, k * 128:(k + 1) * 128], ident)
            nc.scalar.copy(catT[:, k, :], ps)
        psh = psmm.tile([128, 512], F32, tag="mm")
        for k in range(NKC):
            nc.tensor.matmul(psh[:, :H], catT[:, k, :], woutc_sb[:, k, :],
                             start=(k == 0), stop=(k == NKC - 1))
        htilde = work.tile([P, H], F32)
        ht16 = work.tile([P, H], F16)
        nc.scalar.activation(htilde, psh[:, :H], AF_T.Tanh)
        nc.scalar.activation(ht16, psh[:, :H], AF_T.Tanh)
        nc.sync.dma_start(out=O["htilde"], in_=htilde)

        # tgt_k = h_tilde @ Win_k.T
        htT = work.tile([128, NKH, 128], F16)
        for k in range(NKH):
            ps = pstr.tile([128, 128], F16, tag="tr")
            nc.tensor.transpose(ps, ht16[:, k * 128:(k + 1) * 128], ident)
            nc.scalar.copy(htT[:, k, :], ps)
        tgtk16 = work.tile([P, F], F16)
        for (ns, nw) in _split(F):
            ps = psmm.tile([128, 512], F32, tag="mm")
            for k in range(NKH):
                nc.tensor.matmul(ps[:, :nw], htT[:, k, :], wink_sb[:, k, ns:ns + nw],
                                 start=(k == 0), stop=(k == NKH - 1))
            nc.scalar.copy(tgtk16[:, ns:ns + nw], ps[:, :nw])
        stD.close()

        # ---------------- phase E: candidate logits -----------------------
        stF = ExitStack()
        candp = stF.enter_context(tc.tile_pool(name="candp", bufs=2))
        logit = work.tile([P, C], F32)
        scr_k = work.tile([P, F], F16)
        for c in range(C // CH_C):
            cd = candp.tile([P, CH_C, F], F16, tag="cand")
            nc.sync.dma_start(out=cd, in_=I["cand16"][:, c * CH_C:(c + 1) * CH_C, :])
            for j in range(CH_C):
                cc = c * CH_C + j
                nc.vector.scalar_tensor_tensor(
                    out=scr_k, in0=cd[:, j, :], scalar=1.0, in1=tgtk16,
                    op0=OP.bypass, op1=OP.mult, accum_out=logit[:, cc:cc + 1])
        nc.sync.dma_start(out=O["logit"], in_=logit)
        stF.close()


def _declare_io(nc, d):
    P, NV, F, H, E, L, C, AF, KS = (
        d["P"], d["NV"], d["F"], d["H"], d["E"], d["L"], d["C"], d["AF"], d["KS"])
    KXP = _ceil128(E + F)
    di = {}

    def inp(name, shape, dt):
        di[name] = nc.dram_tensor(name, shape, dt, kind="ExternalInput").ap()

    inp("action16", [P, AF], F16)
    inp("prevh16", [P, H], F16)
    inp("c0_32", [P, H], F32)
    inp("maskb32", [P, L], F32)
    inp("feat16", [P, NV, F], F16)
    inp("ctx16", [P, L, 2 * H], F16)
    inp("cand16", [P, C, F], F16)
    inp("wembT", [AF, E], F16)
    inp("bemb", [1, E], F16)
    inp("wfT", [H, F], F16)
    inp("wshT", [H, KS], F16)
    inp("bsh", [1, KS], F16)
    inp("wihT", [KXP, 4 * H], F16)
    inp("whhT", [H, 4 * H], F16)
    inp("bihhh", [1, 4 * H], F16)
    inp("wincT", [H, 2 * H], F16)
    inp("woutcT", [3 * H, H], F16)
    inp("winkT", [H, F], F16)

    do = {}
    for name, shape in [("h1", [P, H]), ("c1", [P, H]), ("logit", [P, C]),
                        ("htilde", [P, H])]:
        do[name] = nc.dram_tensor(name, shape, F32, kind="ExternalOutput").ap()
    return di, do


def _get_compiled(d):
    key = tuple(sorted(d.items()))
    if key in _CACHE:
        return _CACHE[key]
    nc = bacc.Bacc("TRN2", target_bir_lowering=False, debug=False,
                   num_devices=NCORES)
    di, do = _declare_io(nc, d)
    with tile.TileContext(nc) as tc:
        build_graph(tc, d, di, do)
    nc.compile()
    _CACHE[key] = nc
    return nc


def _host_prep(d, action, feature, cand_feat, prev_h1, c_0, ctx, ctx_mask,
               W_emb, b_emb, Win_f, Wsh, bsh, Wih, Whh, bih, bhh,
               Win_c, Wout_c, Win_k):
    """Shard + cast + pre-transpose on the host.  Returns in_maps (one/core)."""
    P = d["P"]
    E, F_, H = d["E"], d["F"], d["H"]
    KXP = _ceil128(E + F_)
    f16 = np.float16
    # shared (replicated) weights
    wihT = np.zeros((KXP, 4 * H), dtype=f16)
    wihT[:E + F_, :] = np.ascontiguousarray(Wih.T).astype(f16)
    shared = {
        "wembT": np.ascontiguousarray(W_emb.T).astype(f16),
        "bemb": b_emb.reshape(1, -1).astype(f16),
        "wfT": np.ascontiguousarray(Win_f.T).astype(f16),
        "wshT": np.ascontiguousarray(Wsh.T).astype(f16),
        "bsh": bsh.reshape(1, -1).astype(f16),
        "wihT": wihT,
        "whhT": np.ascontiguousarray(Whh.T).astype(f16),
        "bihhh": (bih + bhh).reshape(1, -1).astype(f16),
        "wincT": np.ascontiguousarray(Win_c.T).astype(f16),
        "woutcT": np.ascontiguousarray(Wout_c.T).astype(f16),
        "winkT": np.ascontiguousarray(Win_k.T).astype(f16),
    }
    maskb = np.where(ctx_mask != 0, np.float32(-30000.0), np.float32(0.0))
    feat16 = feature.astype(f16)
    ctx16 = ctx.astype(f16)
    cand16 = cand_feat.astype(f16)
    act16 = action.astype(f16)
    ph16 = prev_h1.astype(f16)
    c032 = c_0.astype(np.float32)

    in_maps = []
    for i in range(NCORES):
        sl = slice(i * P, (i + 1) * P)
        m = dict(shared)
        m["action16"] = np.ascontiguousarray(act16[sl])
        m["prevh16"] = np.ascontiguousarray(ph16[sl])
        m["c0_32"] = np.ascontiguousarray(c032[sl])
        m["maskb32"] = np.ascontiguousarray(maskb[sl])
        m["feat16"] = np.ascontiguousarray(feat16[sl])
        m["ctx16"] = np.ascontiguousarray(ctx16[sl])
        m["cand16"] = np.ascontiguousarray(cand16[sl])
        in_maps.append(m)
    return in_maps


def kernel(action, feature, cand_feat, h_0, prev_h1, c_0, ctx, ctx_mask,
           W_emb, b_emb, Win_f, Wsh, bsh, Wih, Whh, bih, bhh,
           Win_c, Wout_c, Win_k):
    global _LAST_RESULT
    d = DIMS
    args = [np.asarray(a) for a in (
        action, feature, cand_feat, prev_h1, c_0, ctx, ctx_mask,
        W_emb, b_emb, Win_f, Wsh, bsh, Wih, Whh, bih, bhh,
        Win_c, Wout_c, Win_k)]
    in_maps = _host_prep(d, *args)
    nc = _get_compiled(d)
    res = run_bass_kernel_spmd(nc, in_maps, core_ids=list(range(NCORES)),
                               trace=_TRACE)
    _LAST_RESULT = res
    outs = res.results
    h_1 = np.concatenate([o["h1"] for o in outs], axis=0)
    c_1 = np.concatenate([o["c1"] for o in outs], axis=0)
    logit = np.concatenate([o["logit"] for o in outs], axis=0)
    h_tilde = np.concatenate([o["htilde"] for o in outs], axis=0)
    return h_1, c_1, logit, h_tilde
